# revision 3
# baseline (speedup 1.0000x reference)
"""Polar encoder (Arikan butterfly) Trainium2 kernel.

Problem structure (hardcoded from the fixed reference):
  - u: [8192, 4096] float32 bits in {0,1}; info_pos = arange(4096, 8192);
    frozen positions 0..4095 -> codeword c = [zeros | u].
  - 13 butterfly stages over N=8192 columns. Because the lower half of c is
    zero, stages 0..11 leave it zero (XOR of zeros) and act on the upper half
    exactly as a 12-stage butterfly over 4096 columns; stage 12 copies the
    upper half into the lower half. So out = [B | B] with
    B = butterfly12(u) over the 4096-column space.
  - Device computes B = [8192, 4096] f32, data-parallel over 8 cores (1024
    rows each); host replicates the column block during unshard.

On-chip compute per 256-row super-tile [128p, 2g, 4096c]:
  f32 -> uint8 cast (ScalarE), then 12 XOR stages on VectorE over int32
  bitcast views (bitwise ops act on raw bytes; each byte is 0/1):
    stage 0: w ^= (w >> 8) & 0x00FF00FF
    stage 1: w ^= (w >> 16)
    stage s>=2: first half of each 2^(s+1)-byte block ^= second half,
                one strided tensor_tensor per stage (2^(s-2)-word runs)
  then uint8 -> f32 cast (ScalarE) and DMA out.
"""
import sys

if "/opt/trn_rl_repo" not in sys.path:
    sys.path.insert(0, "/opt/trn_rl_repo")

import numpy as np

N_CORES = 8
BS = 8192
K = 4096  # info bits per codeword == device-side column count
ROWS_PER_CORE = BS // N_CORES  # 1024
GROUPS = 2  # 128-row groups per super-tile
SUPER = 128 * GROUPS  # 256 rows per super-tile
N_SUPER = ROWS_PER_CORE // SUPER  # 4
WORDS = K // 4  # int32 words per group per partition (1024)

_compiled = None


def _build(repeats=1, timing=False):
    """Build the per-core program.

    timing=True builds a benchmark variant: u/y live in internal DRAM (no
    host transfer) and a tiny token tensor is the only external output, so
    wall-clock differences between repeat counts isolate device exec time.
    """
    from concourse import bacc, mybir, tile

    nc = bacc.Bacc("TRN2", target_bir_lowering=False, debug=False,
                   num_devices=N_CORES)
    if timing:
        u_ap = nc.dram_tensor("u", [ROWS_PER_CORE, K],
                              mybir.dt.float32).ap()
        y_ap = nc.dram_tensor("y", [ROWS_PER_CORE, K],
                              mybir.dt.float32).ap()
        tok_ap = nc.dram_tensor("tok", [128, 4], mybir.dt.float32,
                                kind="ExternalOutput").ap()
    else:
        u_ap = nc.dram_tensor("u", [ROWS_PER_CORE, K], mybir.dt.float32,
                              kind="ExternalInput").ap()
        y_ap = nc.dram_tensor("y", [ROWS_PER_CORE, K], mybir.dt.float32,
                              kind="ExternalOutput").ap()
    xor = mybir.AluOpType.bitwise_xor

    with tile.TileContext(nc) as tc:
        with tc.tile_pool(name="sbuf", bufs=2) as pool:
            for it in range(repeats * N_SUPER):
                t = it % N_SUPER
                r0 = t * SUPER
                src = u_ap[r0:r0 + SUPER, :].rearrange(
                    "(g p) c -> p g c", p=128)
                dst = y_ap[r0:r0 + SUPER, :].rearrange(
                    "(g p) c -> p g c", p=128)

                t_in = pool.tile([128, GROUPS, K], mybir.dt.float32)
                nc.sync.dma_start(out=t_in[:], in_=src)

                t_u8 = pool.tile([128, GROUPS, K], mybir.dt.uint8)
                nc.scalar.copy(out=t_u8[:], in_=t_in[:])

                w = t_u8[:].bitcast(mybir.dt.int32)  # [128, GROUPS, WORDS]
                t_tmp = pool.tile([128, GROUPS, WORDS], mybir.dt.int32)

                # stage 0: bytes (0,2) ^= bytes (1,3) within each word
                nc.vector.tensor_scalar(
                    out=t_tmp[:], in0=w, scalar1=8, scalar2=0x00FF00FF,
                    op0=mybir.AluOpType.logical_shift_right,
                    op1=mybir.AluOpType.bitwise_and)
                nc.vector.tensor_tensor(out=w, in0=w, in1=t_tmp[:], op=xor)

                # stage 1: bytes (0,1) ^= bytes (2,3); zero-fill needs no mask
                nc.vector.tensor_scalar(
                    out=t_tmp[:], in0=w, scalar1=16, scalar2=None,
                    op0=mybir.AluOpType.logical_shift_right)
                nc.vector.tensor_tensor(out=w, in0=w, in1=t_tmp[:], op=xor)

                # stages 2..11: halves of 2R-word blocks
                for s in range(2, 12):
                    run = 1 << (s - 2)  # words per half-block
                    nb = WORDS // (2 * run)
                    v = w.rearrange("p g (nb two r) -> p g nb two r",
                                    two=2, r=run)
                    nc.vector.tensor_tensor(
                        out=v[:, :, :, 0:1, :], in0=v[:, :, :, 0:1, :],
                        in1=v[:, :, :, 1:2, :], op=xor)

                t_out = pool.tile([128, GROUPS, K], mybir.dt.float32)
                nc.scalar.copy(out=t_out[:], in_=t_u8[:])
                nc.scalar.dma_start(out=dst, in_=t_out[:])

            if timing:
                t_last = pool.tile([128, 4], mybir.dt.float32)
                nc.vector.tensor_copy(out=t_last[:], in_=t_out[:, 0, 0:4])
                nc.sync.dma_start(out=tok_ap[:], in_=t_last[:])

    nc.compile()
    return nc


def _reference_fallback(u, info_pos, ind_gather):
    """Generic numpy path, used only if the input structure ever deviates
    from the fixed reference layout this kernel hardcodes."""
    bs = u.shape[0]
    n = ind_gather.shape[1] - 1
    c = np.zeros((bs, n), dtype=u.dtype)
    c[:, np.asarray(info_pos)] = u
    x = np.concatenate([c, np.zeros((bs, 1), dtype=u.dtype)], axis=1)
    for s in range(ind_gather.shape[0]):
        x = (x + x[:, np.asarray(ind_gather[s])]) % 2
    return x[:, :n]


def kernel(u, info_pos, ind_gather):
    global _compiled
    u = np.asarray(u)
    expected_structure = (
        u.shape == (BS, K)
        and np.array_equal(np.asarray(info_pos), np.arange(K, BS, dtype=np.int32))
    )
    if not expected_structure:
        return _reference_fallback(u, info_pos, ind_gather)

    from concourse.bass_utils import run_bass_kernel_spmd

    if _compiled is None:
        _compiled = _build()

    in_maps = [
        {"u": u[i * ROWS_PER_CORE:(i + 1) * ROWS_PER_CORE]}
        for i in range(N_CORES)
    ]
    res = run_bass_kernel_spmd(_compiled, in_maps, list(range(N_CORES)))
    b = np.concatenate([res.results[i]["y"] for i in range(N_CORES)], axis=0)
    return np.concatenate([b, b], axis=1)
